# revision 2
# baseline (speedup 1.0000x reference)
"""Trainium2 Bass kernel: 2-layer Chebyshev graph conv (K=5) + 3-layer MLP head.

Distribution over 8 NeuronCores (v2 — SBUF-resident adjacency):
  - The adjacency a [8192, 8192] is row-sharded; core i owns graph rows
    [1024*i, 1024*(i+1)), passed host-transposed AND host-cast to bf16
    (at: [8192, 1024]) so the contraction dim m is the partition dim of the
    stationary matmul operand.  The whole 16 MiB shard is loaded into SBUF
    ONCE and reused by all 8 Chebyshev applications (4 in conv1, 4 in
    conv2) — the baseline re-streamed 32 MiB fp32 per application.
  - Chebyshev states are node-major bf16.  Per application, each core
    computes its local 1024 rows (psum accumulation over 64 m-chunks with
    the at-slices stationary), then AllGathers them for the next
    application.  conv2 runs in two j-halves (4 PSUM banks each) so the
    combine/spill/gather of one half pipelines under the other's matmuls.
  - conv outputs accumulate in f32 SBUF via per-j PE transposes +
    block-diagonal weight matmuls (weights bf16).
  - fc1 is contraction(row)-sharded over nodes, weights streamed as bf16;
    partial [16, 512] results are AllReduced, then fc2/fc3/softmax run
    redundantly on every core.
"""

import os
import sys

import numpy as np

for _p in ("/opt/trn_rl_repo", "/root/.axon_site/_ro/trn_rl_repo"):
    if os.path.isdir(_p) and _p not in sys.path:
        sys.path.insert(0, _p)

P = 128          # SBUF partitions
N = 8192         # nodes
B = 16           # batch
F_IN = 2
F1 = 32
F2 = 32
K = 5            # Chebyshev order
NCORES = 8
R = N // NCORES  # nodes per core (1024)
MC = N // P      # m-chunks (64)
NJ = R // P      # local n-chunks (8)
S1 = B * F_IN    # conv1 state width (32)
C2 = B * F1      # 512
M1, M2, M3 = 512, 128, 2
HJ = NJ // 2     # j-chunks per half (4)

REPEAT = int(os.environ.get("KER_REPEAT", "1"))    # emit the body N times (timing A/B)

_CACHE = {}


def build_kernel(dt_graph=None, dt_fc1=None, repeat=None):
    from concourse import bacc, mybir, tile
    from concourse.masks import make_identity

    REPEAT = repeat if repeat is not None else globals()["REPEAT"]

    dt = mybir.dt
    f32 = dt.float32
    f32r = dt.float32r
    bf16 = dt.bfloat16
    Alu = mybir.AluOpType
    Act = mybir.ActivationFunctionType
    RG = [list(range(NCORES))]

    nc = bacc.Bacc(
        "TRN2",
        target_bir_lowering=False,
        debug=False,
        enable_asserts=False,
        num_devices=NCORES,
    )

    # ------------------------- DRAM I/O -------------------------
    at_d = nc.dram_tensor("at", [N, R], bf16, kind="ExternalInput").ap()
    x2dp_d = nc.dram_tensor("x2dp", [N, S1], bf16, kind="ExternalInput").ap()
    xloc_d = nc.dram_tensor("xloc", [R, S1], bf16, kind="ExternalInput").ap()
    xlocT_d = nc.dram_tensor("xlocT", [S1, R], bf16, kind="ExternalInput").ap()
    w1bd_d = nc.dram_tensor("w1bd", [K, S1, C2], bf16, kind="ExternalInput").ap()
    w2bd_d = nc.dram_tensor("w2bd", [K, P, P], bf16, kind="ExternalInput").ap()
    fw1s_d = nc.dram_tensor("fw1s", [R * F2, M1], bf16, kind="ExternalInput").ap()
    fw2_d = nc.dram_tensor("fw2", [M1, M2], f32r, kind="ExternalInput").ap()
    fw3_d = nc.dram_tensor("fw3", [M2, M3], f32r, kind="ExternalInput").ap()
    b1r_d = nc.dram_tensor("b1r", [P, C2], f32, kind="ExternalInput").ap()
    b2r_d = nc.dram_tensor("b2r", [P, C2], f32, kind="ExternalInput").ap()
    fb1r_d = nc.dram_tensor("fb1r", [B, M1], f32, kind="ExternalInput").ap()
    fb2r_d = nc.dram_tensor("fb2r", [B, M2], f32, kind="ExternalInput").ap()
    fb3r_d = nc.dram_tensor("fb3r", [B, M3], f32, kind="ExternalInput").ap()
    out_d = nc.dram_tensor("out", [B, M3], f32, kind="ExternalOutput").ap()

    with tile.TileContext(nc) as tc:
        with (
            tc.tile_pool(name="consts", bufs=1) as consts,
            tc.tile_pool(name="tf1p", bufs=2) as tf1p,
            tc.tile_pool(name="tf2p", bufs=6) as tf2p,
            tc.tile_pool(name="accp", bufs=8) as accp,
            tc.tile_pool(name="tl1p", bufs=16) as tl1p,
            tc.tile_pool(name="tl2p", bufs=16) as tl2p,
            tc.tile_pool(name="ttp", bufs=4) as ttp,
            tc.tile_pool(name="fwp", bufs=6) as fwp,
            tc.tile_pool(name="fcp", bufs=2) as fcp,
            tc.tile_pool(name="psum", bufs=1, space="PSUM") as psp,
            tc.tile_pool(name="dram", bufs=2, space="DRAM") as drp,
        ):
            # ------------------------- constants -------------------------
            identb = consts.tile([P, P], bf16)
            make_identity(nc, identb)
            identf = consts.tile([P, P], f32)
            make_identity(nc, identf)
            # the full a^T shard, SBUF-resident: [m-partition, mc, r]
            atsb = consts.tile([P, MC * R], bf16)
            nc.sync.dma_start(
                out=atsb.rearrange("p (mc r) -> p mc r", r=R),
                in_=at_d.rearrange("(mc p) r -> p mc r", p=P),
            )
            w1sb = consts.tile([S1, K * C2], bf16)
            for k in range(K):
                nc.scalar.dma_start(out=w1sb[:, k * C2:(k + 1) * C2], in_=w1bd_d[k])
            w2sb = consts.tile([P, K * P], bf16)
            for k in range(K):
                nc.scalar.dma_start(out=w2sb[:, k * P:(k + 1) * P], in_=w2bd_d[k])
            xlT = consts.tile([S1, R], bf16)
            nc.scalar.dma_start(out=xlT[:], in_=xlocT_d[:])
            b1r = consts.tile([P, C2], f32)
            nc.scalar.dma_start(out=b1r[:], in_=b1r_d[:])
            b2r = consts.tile([P, C2], f32)
            nc.scalar.dma_start(out=b2r[:], in_=b2r_d[:])
            fb1r = consts.tile([B, M1], f32)
            nc.scalar.dma_start(out=fb1r[:], in_=fb1r_d[:])
            fb2r = consts.tile([B, M2], f32)
            nc.scalar.dma_start(out=fb2r[:], in_=fb2r_d[:])
            fb3r = consts.tile([B, M3], f32)
            nc.scalar.dma_start(out=fb3r[:], in_=fb3r_d[:])
            fw3sb = consts.tile([M2, M3], f32r)
            nc.scalar.dma_start(out=fw3sb[:], in_=fw3_d[:])

            def at_sl(mc, j):
                return atsb[:, mc * R + j * P:mc * R + (j + 1) * P]

            def emit_body(rep):
                # =============================================================
                # conv1 — node-major states [P, S1], at stationary
                # =============================================================
                out1 = []
                for j in range(NJ):
                    t = accp.tile([P, C2], f32, tag="acc", name=f"out1_{rep}_{j}")
                    nc.vector.memset(t[:], 0.0)
                    out1.append(t)

                def conv1_feature(k, get_lhsT):
                    """out1[j] += lhsT_j.T @ w1bd[k]; lhsT_j: [S1, P] bf16."""
                    for j in range(NJ):
                        f_ps = psp.tile(
                            [P, C2], f32, tag="psff", bufs=1, name=f"c1f_{rep}_{k}_{j}"
                        )
                        nc.tensor.matmul(
                            f_ps[:], get_lhsT(j), w1sb[:, k * C2:(k + 1) * C2],
                            start=True, stop=True,
                        )
                        nc.vector.tensor_add(out1[j][:], out1[j][:], f_ps[:])

                def conv1_tr(dst, k):
                    """Build transposed locals [S1, P] for the feature matmul."""
                    tts = []
                    for j in range(NJ):
                        psT = psp.tile(
                            [S1, P], f32, tag="psf", bufs=2, name=f"c1T_{rep}_{k}_{j}"
                        )
                        nc.tensor.transpose(psT[:], dst[j][:], identb[:])
                        tt = ttp.tile([S1, P], bf16, tag="tt1", name=f"c1tt_{rep}_{k}_{j}")
                        nc.vector.tensor_copy(tt[:], psT[:])
                        tts.append(tt)
                    return tts

                # T0 = x: full node-major copy + local tiles
                tf1 = tf1p.tile([P, MC * S1], bf16, tag="tf1", name=f"tf1_{rep}_t0")
                nc.scalar.dma_start(
                    out=tf1.rearrange("p (mc s) -> p mc s", s=S1),
                    in_=x2dp_d.rearrange("(mc p) s -> p mc s", p=P),
                )
                tlA = []
                for j in range(NJ):
                    t = tl1p.tile([P, S1], bf16, tag="tl1", name=f"tlA_{rep}_{j}")
                    nc.scalar.dma_start(out=t[:], in_=xloc_d[j * P:(j + 1) * P, :])
                    tlA.append(t)
                tlB = [
                    tl1p.tile([P, S1], bf16, tag="tl1", name=f"tlB_{rep}_{j}")
                    for j in range(NJ)
                ]
                conv1_feature(0, lambda j: xlT[:, j * P:(j + 1) * P])

                src1 = x2dp_d
                for k in range(1, K):
                    # apply: T*_local = a @ T_{k-1}, two j-halves of 4 psum tiles
                    if k > 1:
                        tf1 = tf1p.tile(
                            [P, MC * S1], bf16, tag="tf1", name=f"tf1_{rep}_{k}"
                        )
                        nc.scalar.dma_start(
                            out=tf1.rearrange("p (mc s) -> p mc s", s=S1),
                            in_=src1.rearrange("(mc p) s -> p mc s", p=P),
                        )
                    dst = tlB if k % 2 == 1 else tlA
                    for h in range(2):
                        ps1 = [
                            psp.tile(
                                [P, S1], f32, tag="psa", bufs=4,
                                name=f"c1ps_{rep}_{k}_{h}_{j}",
                            )
                            for j in range(HJ)
                        ]
                        for mc in range(MC):
                            tfc = tf1[:, mc * S1:(mc + 1) * S1]
                            for jj in range(HJ):
                                nc.tensor.matmul(
                                    ps1[jj][:],
                                    at_sl(mc, h * HJ + jj),
                                    tfc,
                                    start=(mc == 0),
                                    stop=(mc == MC - 1),
                                )
                        # combine: T_k = 2 a T_{k-1} - T_{k-2} (bf16 out)
                        for jj in range(HJ):
                            j = h * HJ + jj
                            if k == 1:
                                nc.vector.tensor_copy(dst[j][:], ps1[jj][:])
                            else:
                                nc.vector.scalar_tensor_tensor(
                                    dst[j][:], ps1[jj][:], 2.0, dst[j][:],
                                    Alu.mult, Alu.subtract,
                                )
                    # gather T_k for the next application
                    if k < K - 1:
                        cc_in = drp.tile([R, S1], bf16, tag="cc1i", name=f"cc1i_{rep}_{k}")
                        for j in range(NJ):
                            nc.sync.dma_start(
                                out=cc_in[j * P:(j + 1) * P, :], in_=dst[j][:]
                            )
                        cc_out = drp.tile(
                            [N, S1], bf16, tag="cc1o", addr_space="Shared",
                            name=f"cc1o_{rep}_{k}",
                        )
                        nc.gpsimd.collective_compute(
                            "AllGather", Alu.bypass, replica_groups=RG,
                            ins=[cc_in.opt()], outs=[cc_out.opt()],
                        )
                        src1 = cc_out
                    conv1_feature(k, (lambda ts: (lambda j: ts[j][:]))(conv1_tr(dst, k)))

                # conv1 epilogue: h = relu(out1 + b1) -> bf16; gather h
                hb = []
                for j in range(NJ):
                    nc.vector.tensor_add(out1[j][:], out1[j][:], b1r[:])
                    nc.vector.tensor_relu(out1[j][:], out1[j][:])
                    t = tl2p.tile([P, C2], bf16, tag="tl2", name=f"hb_{rep}_{j}")
                    nc.vector.tensor_copy(t[:], out1[j][:])
                    hb.append(t)
                cc_hi = drp.tile([R, C2], bf16, tag="cc2i", name=f"cc_hi_{rep}")
                for j in range(NJ):
                    nc.sync.dma_start(out=cc_hi[j * P:(j + 1) * P, :], in_=hb[j][:])
                cc_h = drp.tile(
                    [N, C2], bf16, tag="cc2o", addr_space="Shared", name=f"cc_h_{rep}"
                )
                nc.gpsimd.collective_compute(
                    "AllGather", Alu.bypass, replica_groups=RG,
                    ins=[cc_hi.opt()], outs=[cc_h.opt()],
                )

                # =============================================================
                # conv2 — states [P, C2] bf16, pipelined j-halves
                # =============================================================
                out2 = []
                for j in range(NJ):
                    t = accp.tile([P, C2], f32, tag="acc", name=f"out2_{rep}_{j}")
                    nc.vector.memset(t[:], 0.0)
                    out2.append(t)

                def conv2_feature(tk_set, k, js):
                    """out2[j] += sum_c T_k[j][:, c].T @ w2bd[k] (block-diag)."""
                    for j in js:
                        f_ps = psp.tile(
                            [P, C2], f32, tag="psff", bufs=1,
                            name=f"c2f_{rep}_{k}_{j}",
                        )
                        for c in range(4):
                            tt_ps = psp.tile(
                                [P, P], f32, tag="psf", bufs=2,
                                name=f"c2T_{rep}_{k}_{j}_{c}",
                            )
                            nc.tensor.transpose(
                                tt_ps[:], tk_set[j][:, c * P:(c + 1) * P], identb[:]
                            )
                            tt = ttp.tile(
                                [P, P], bf16, tag="tt2", name=f"c2tt_{rep}_{k}_{j}_{c}"
                            )
                            nc.vector.tensor_copy(tt[:], tt_ps[:])
                            nc.tensor.matmul(
                                f_ps[:, c * P:(c + 1) * P],
                                tt[:],
                                w2sb[:, k * P:(k + 1) * P],
                                start=True,
                                stop=True,
                            )
                        nc.vector.tensor_add(out2[j][:], out2[j][:], f_ps[:])

                tlB2 = [
                    tl2p.tile([P, C2], bf16, tag="tl2", name=f"tlB2_{rep}_{j}")
                    for j in range(NJ)
                ]
                conv2_feature(hb, 0, range(NJ))  # T0 = h (local rows in hb)

                # gathered T chunks: chunk mc lives in the half-gather of
                # core mc//8, half (mc%8)//4, at rows (mc//8)*512+(mc%4)*128
                srcs = [cc_h, cc_h]

                def tft_src(k, mc):
                    if k == 1:
                        return cc_h[mc * P:(mc + 1) * P, :]
                    c, rem = divmod(mc, NJ)
                    hh, loc = divmod(rem, HJ)
                    off = c * (R // 2) + loc * P
                    return srcs[hh][off:off + P, :]

                for k in range(1, K):
                    dst = tlB2 if k % 2 == 1 else hb
                    new_srcs = [None, None]
                    for h in range(2):
                        ps_g = [
                            psp.tile(
                                [P, C2], f32, tag="psa", bufs=4,
                                name=f"c2g_{rep}_{k}_{h}_{j}",
                            )
                            for j in range(HJ)
                        ]
                        for mc in range(MC):
                            tft = tf2p.tile(
                                [P, C2], bf16, tag="tf2", name=f"tf2_{rep}_{k}_{h}_{mc}"
                            )
                            nc.scalar.dma_start(out=tft[:], in_=tft_src(k, mc))
                            for jj in range(HJ):
                                nc.tensor.matmul(
                                    ps_g[jj][:],
                                    at_sl(mc, h * HJ + jj),
                                    tft[:],
                                    start=(mc == 0),
                                    stop=(mc == MC - 1),
                                )
                        for jj in range(HJ):
                            j = h * HJ + jj
                            if k == 1:
                                nc.vector.tensor_copy(dst[j][:], ps_g[jj][:])
                            else:
                                nc.vector.scalar_tensor_tensor(
                                    dst[j][:], ps_g[jj][:], 2.0, dst[j][:],
                                    Alu.mult, Alu.subtract,
                                )
                        if k < K - 1:
                            cc_in = drp.tile(
                                [R // 2, C2], bf16, tag="cc2i",
                                name=f"cc2i_{rep}_{k}_{h}",
                            )
                            for jj in range(HJ):
                                nc.sync.dma_start(
                                    out=cc_in[jj * P:(jj + 1) * P, :],
                                    in_=dst[h * HJ + jj][:],
                                )
                            cc_out = drp.tile(
                                [N // 2, C2], bf16, tag="cc2o", addr_space="Shared",
                                name=f"cc2o_{rep}_{k}_{h}",
                            )
                            nc.gpsimd.collective_compute(
                                "AllGather", Alu.bypass, replica_groups=RG,
                                ins=[cc_in.opt()], outs=[cc_out.opt()],
                            )
                            new_srcs[h] = cc_out
                        conv2_feature(dst, k, range(h * HJ, (h + 1) * HJ))
                    srcs = new_srcs

                # conv2 epilogue: h2 = relu(out2 + b2) -> bf16 fc1 lhs
                fc_lhs = []
                for j in range(NJ):
                    nc.vector.tensor_add(out2[j][:], out2[j][:], b2r[:])
                    nc.vector.tensor_relu(out2[j][:], out2[j][:])
                    t = tl2p.tile([P, C2], bf16, tag="tl2", name=f"h2b_{rep}_{j}")
                    nc.vector.tensor_copy(t[:], out2[j][:])
                    fc_lhs.append(t)

                # =============================================================
                # fc1 (node-sharded contraction) + AllReduce
                # =============================================================
                fw1v = fw1s_d.rearrange("(j p f) m -> j f p m", p=P, f=F2)
                fc_ps = psp.tile([B, M1], f32, tag="psfc", bufs=1, name=f"fc1_{rep}")
                n_mm = NJ * F2
                i_mm = 0
                for j in range(NJ):
                    lhs_j = fc_lhs[j].rearrange("p (b f) -> p f b", f=F2)
                    for f in range(F2):
                        fwt = fwp.tile(
                            [P, M1], bf16, tag="fw", name=f"fw1_{rep}_{j}_{f}"
                        )
                        eng = nc.sync if f % 2 == 0 else nc.scalar
                        eng.dma_start(out=fwt[:], in_=fw1v[j, f])
                        nc.tensor.matmul(
                            fc_ps[:],
                            lhs_j[:, f, :],
                            fwt[:],
                            start=(i_mm == 0),
                            stop=(i_mm == n_mm - 1),
                        )
                        i_mm += 1

                z = fcp.tile([B, M1], f32, tag="z", name=f"z_{rep}")
                nc.vector.tensor_copy(z[:], fc_ps[:])
                cc_fi = drp.tile([B, M1], f32, tag="ccfi", name=f"ccfi_{rep}")
                nc.sync.dma_start(out=cc_fi[:], in_=z[:])
                cc_fo = drp.tile(
                    [B, M1], f32, tag="ccfo", addr_space="Shared", name=f"ccfo_{rep}"
                )
                nc.gpsimd.collective_compute(
                    "AllReduce", Alu.add, replica_groups=RG,
                    ins=[cc_fi.opt()], outs=[cc_fo.opt()],
                )

                # z1 = relu(fc1 + fb1), padded to 32 partitions for PE transpose
                z1p = fcp.tile([32, M1], f32, tag="z1p", name=f"z1p_{rep}")
                nc.vector.memset(z1p[:], 0.0)
                nc.sync.dma_start(out=z1p[0:B, :], in_=cc_fo[:])
                nc.vector.tensor_add(z1p[0:B, :], z1p[0:B, :], fb1r[:])
                nc.vector.tensor_relu(z1p[0:B, :], z1p[0:B, :])

                # fc2
                fc2_ps = psp.tile([B, M2], f32, tag="psfc", bufs=1, name=f"fc2_{rep}")
                for c in range(4):
                    zt_ps = psp.tile(
                        [P, 32], f32, tag="psf", bufs=2, name=f"ztps_{rep}_{c}"
                    )
                    nc.tensor.transpose(
                        zt_ps[:], z1p[:, c * P:(c + 1) * P], identf[0:32, 0:32]
                    )
                    zt = fcp.tile([P, 32], f32r, tag="zt", name=f"zt_{rep}_{c}")
                    nc.vector.tensor_copy(zt[:], zt_ps[:])
                    fwt2 = fcp.tile([P, M2], f32r, tag="fw2t", name=f"fw2t_{rep}_{c}")
                    nc.sync.dma_start(out=fwt2[:], in_=fw2_d[c * P:(c + 1) * P, :])
                    nc.tensor.matmul(
                        fc2_ps[:],
                        zt[:, 0:B],
                        fwt2[:],
                        start=(c == 0),
                        stop=(c == 3),
                    )
                z2p = fcp.tile([32, M2], f32, tag="z2p", name=f"z2p_{rep}")
                nc.vector.memset(z2p[:], 0.0)
                nc.vector.tensor_copy(z2p[0:B, :], fc2_ps[:])
                nc.vector.tensor_add(z2p[0:B, :], z2p[0:B, :], fb2r[:])
                nc.vector.tensor_relu(z2p[0:B, :], z2p[0:B, :])

                # fc3
                z3t_ps = psp.tile([P, 32], f32, tag="psf", bufs=2, name=f"z3tps_{rep}")
                nc.tensor.transpose(z3t_ps[:], z2p[:], identf[0:32, 0:32])
                z3t = fcp.tile([P, 32], f32r, tag="z3t", name=f"z3t_{rep}")
                nc.vector.tensor_copy(z3t[:], z3t_ps[:])
                fc3_ps = psp.tile([B, M3], f32, tag="psf", bufs=2, name=f"fc3_{rep}")
                nc.tensor.matmul(
                    fc3_ps[:], z3t[:, 0:B], fw3sb[:], start=True, stop=True
                )
                s = fcp.tile([B, M3], f32, tag="s", name=f"s_{rep}")
                nc.vector.tensor_copy(s[:], fc3_ps[:])
                nc.vector.tensor_add(s[:], s[:], fb3r[:])

                # softmax over the last dim (M3 = 2)
                mx = fcp.tile([B, 1], f32, tag="mx", name=f"mx_{rep}")
                nc.vector.reduce_max(mx[:], s[:], axis=mybir.AxisListType.X)
                nc.vector.tensor_scalar_mul(mx[:], mx[:], -1.0)
                nc.scalar.activation(s[:], s[:], Act.Exp, bias=mx[:, 0:1])
                sm = fcp.tile([B, 1], f32, tag="sm", name=f"sm_{rep}")
                nc.vector.reduce_sum(sm[:], s[:], axis=mybir.AxisListType.X)
                nc.vector.reciprocal(sm[:], sm[:])
                nc.vector.tensor_scalar_mul(s[:], s[:], sm[:, 0:1])
                nc.sync.dma_start(out=out_d[:], in_=s[:])

            for _rep in range(REPEAT):
                emit_body(_rep)

    nc.compile()
    return nc


def prepare_inputs(x, a, w1, b1, w2, b2, fw1, fb1, fw2, fb2, fw3, fb3):
    """Shard + re-layout the full model inputs into 8 per-core input maps."""
    import ml_dtypes

    bf = ml_dtypes.bfloat16

    x = np.asarray(x, np.float32)
    a = np.asarray(a, np.float32)
    w1 = np.asarray(w1, np.float32)
    w2 = np.asarray(w2, np.float32)
    fw1 = np.asarray(fw1, np.float32)

    # node-major [N, B*F_IN]
    x2d = x.transpose(1, 0, 2).reshape(N, S1)
    x2d_c = x2d.astype(bf)

    w1bd = np.zeros((K, S1, C2), np.float32)
    for b in range(B):
        w1bd[:, b * F_IN:(b + 1) * F_IN, b * F1:(b + 1) * F1] = w1
    w1bd_c = w1bd.astype(bf)
    w2bd = np.zeros((K, P, P), np.float32)
    for q in range(4):
        w2bd[:, q * F1:(q + 1) * F1, q * F2:(q + 1) * F2] = w2
    w2bd_c = w2bd.astype(bf)

    b1r = np.broadcast_to(np.tile(np.asarray(b1, np.float32), B), (P, C2)).copy()
    b2r = np.broadcast_to(np.tile(np.asarray(b2, np.float32), B), (P, C2)).copy()
    fb1r = np.broadcast_to(np.asarray(fb1, np.float32), (B, M1)).copy()
    fb2r = np.broadcast_to(np.asarray(fb2, np.float32), (B, M2)).copy()
    fb3r = np.broadcast_to(np.asarray(fb3, np.float32), (B, M3)).copy()
    fw2_c = np.asarray(fw2, np.float32)
    fw3_c = np.asarray(fw3, np.float32)

    fw1_3 = fw1.reshape(N, F2, M1)

    in_maps = []
    for i in range(NCORES):
        r0 = i * R
        at_i = np.ascontiguousarray(a[r0:r0 + R, :].T).astype(bf)
        xloc = np.ascontiguousarray(x2d[r0:r0 + R]).astype(bf)
        xlocT = np.ascontiguousarray(x2d[r0:r0 + R].T).astype(bf)
        fw1s = np.ascontiguousarray(fw1_3[r0:r0 + R].reshape(R * F2, M1)).astype(bf)
        in_maps.append(
            {
                "at": at_i,
                "x2dp": x2d_c,
                "xloc": xloc,
                "xlocT": xlocT,
                "w1bd": w1bd_c,
                "w2bd": w2bd_c,
                "fw1s": fw1s,
                "fw2": fw2_c,
                "fw3": fw3_c,
                "b1r": b1r,
                "b2r": b2r,
                "fb1r": fb1r,
                "fb2r": fb2r,
                "fb3r": fb3r,
            }
        )
    return in_maps


def kernel(**inputs) -> np.ndarray:
    from concourse.bass_utils import run_bass_kernel_spmd

    key = "v2"
    if key not in _CACHE:
        _CACHE[key] = build_kernel()
    nc = _CACHE[key]

    in_maps = prepare_inputs(**inputs)
    res = run_bass_kernel_spmd(nc, in_maps, core_ids=list(range(NCORES)))
    return np.asarray(res.results[0]["out"], np.float32)


if __name__ == "__main__":
    import importlib.util

    spec = importlib.util.spec_from_file_location(
        "reference", os.path.join(os.path.dirname(__file__), "reference.py")
    )
    ref = importlib.util.module_from_spec(spec)
    spec.loader.exec_module(ref)
    inputs = {k: np.asarray(v) for k, v in ref.setup_inputs().items()}
    out = kernel(**inputs)
    print(out)


# revision 6
# speedup vs baseline: 3.8310x; 3.8310x over previous
"""Trainium2 Bass kernel: 2-layer Chebyshev graph conv (K=5) + 3-layer MLP head.

Distribution over 8 NeuronCores (v2 — SBUF-resident adjacency):
  - The adjacency a [8192, 8192] is row-sharded; core i owns graph rows
    [1024*i, 1024*(i+1)), passed host-transposed AND host-cast to bf16
    (at: [8192, 1024]) so the contraction dim m is the partition dim of the
    stationary matmul operand.  The whole 16 MiB shard is loaded into SBUF
    ONCE and reused by all 8 Chebyshev applications (4 in conv1, 4 in
    conv2) — the baseline re-streamed 32 MiB fp32 per application.
  - Chebyshev states are node-major bf16.  Per application, each core
    computes its local 1024 rows (psum accumulation over 64 m-chunks with
    the at-slices stationary), then AllGathers them for the next
    application.  conv2 runs in two j-halves (4 PSUM banks each) so the
    combine/spill/gather of one half pipelines under the other's matmuls.
  - conv outputs accumulate in f32 SBUF via per-j PE transposes +
    block-diagonal weight matmuls (weights bf16).
  - fc1 is contraction(row)-sharded over nodes, weights streamed as bf16;
    partial [16, 512] results are AllReduced, then fc2/fc3/softmax run
    redundantly on every core.
"""

import os
import sys

import numpy as np

for _p in ("/opt/trn_rl_repo", "/root/.axon_site/_ro/trn_rl_repo"):
    if os.path.isdir(_p) and _p not in sys.path:
        sys.path.insert(0, _p)

P = 128          # SBUF partitions
N = 8192         # nodes
B = 16           # batch
F_IN = 2
F1 = 32
F2 = 32
K = 5            # Chebyshev order
NCORES = 8
R = N // NCORES  # nodes per core (1024)
MC = N // P      # m-chunks (64)
NJ = R // P      # local n-chunks (8)
S1 = B * F_IN    # conv1 state width (32)
C2 = B * F1      # 512
M1, M2, M3 = 512, 128, 2
HJ = NJ // 2     # j-chunks per half (4)

REPEAT = int(os.environ.get("KER_REPEAT", "1"))    # emit the body N times (timing A/B)

_CACHE = {}


def build_kernel(dt_graph=None, dt_fc1=None, repeat=None):
    from concourse import bacc, mybir, tile
    from concourse.masks import make_identity

    REPEAT = repeat if repeat is not None else globals()["REPEAT"]

    dt = mybir.dt
    f32 = dt.float32
    f32r = dt.float32r
    bf16 = dt.bfloat16
    Alu = mybir.AluOpType
    Act = mybir.ActivationFunctionType
    RG = [list(range(NCORES))]

    nc = bacc.Bacc(
        "TRN2",
        target_bir_lowering=False,
        debug=False,
        enable_asserts=False,
        num_devices=NCORES,
    )

    # ------------------------- DRAM I/O -------------------------
    at_d = nc.dram_tensor("at", [N, R], bf16, kind="ExternalInput").ap()
    x2dp_d = nc.dram_tensor("x2dp", [N, S1], bf16, kind="ExternalInput").ap()
    xloc_d = nc.dram_tensor("xloc", [R, S1], bf16, kind="ExternalInput").ap()
    xlocT_d = nc.dram_tensor("xlocT", [S1, R], bf16, kind="ExternalInput").ap()
    w1bd_d = nc.dram_tensor("w1bd", [K, S1, C2], bf16, kind="ExternalInput").ap()
    w2bd_d = nc.dram_tensor("w2bd", [K, P, P], bf16, kind="ExternalInput").ap()
    fw1s_d = nc.dram_tensor("fw1s", [R * F2, M1], bf16, kind="ExternalInput").ap()
    fw2_d = nc.dram_tensor("fw2", [M1, M2], f32r, kind="ExternalInput").ap()
    fw3_d = nc.dram_tensor("fw3", [M2, M3], f32r, kind="ExternalInput").ap()
    b1r_d = nc.dram_tensor("b1r", [P, C2], f32, kind="ExternalInput").ap()
    b2r_d = nc.dram_tensor("b2r", [P, C2], f32, kind="ExternalInput").ap()
    fb1r_d = nc.dram_tensor("fb1r", [B, M1], f32, kind="ExternalInput").ap()
    fb2r_d = nc.dram_tensor("fb2r", [B, M2], f32, kind="ExternalInput").ap()
    fb3r_d = nc.dram_tensor("fb3r", [B, M3], f32, kind="ExternalInput").ap()
    out_d = nc.dram_tensor("out", [B, M3], f32, kind="ExternalOutput").ap()

    with tile.TileContext(nc) as tc:
        with (
            tc.tile_pool(name="consts", bufs=1) as consts,
            tc.tile_pool(name="tf1p", bufs=2) as tf1p,
            tc.tile_pool(name="tf2p", bufs=6) as tf2p,
            tc.tile_pool(name="accp", bufs=8) as accp,
            tc.tile_pool(name="tl1p", bufs=16) as tl1p,
            tc.tile_pool(name="tl2p", bufs=16) as tl2p,
            tc.tile_pool(name="ttp", bufs=4) as ttp,
            tc.tile_pool(name="fwp", bufs=6) as fwp,
            tc.tile_pool(name="fcp", bufs=1) as fcp,
            tc.tile_pool(name="psum", bufs=1, space="PSUM") as psp,
            tc.tile_pool(name="dram", bufs=3, space="DRAM") as drp,
        ):
            # ------------------------- constants -------------------------
            identb = consts.tile([P, P], bf16)
            make_identity(nc, identb)
            identf = consts.tile([P, P], f32)
            make_identity(nc, identf)
            # the full a^T shard, SBUF-resident: [m-partition, mc, r]
            atsb = consts.tile([P, MC * R], bf16)
            nc.sync.dma_start(
                out=atsb.rearrange("p (mc r) -> p mc r", r=R),
                in_=at_d.rearrange("(mc p) r -> p mc r", p=P),
            )
            w1sb = consts.tile([S1, K * C2], bf16)
            for k in range(K):
                nc.scalar.dma_start(out=w1sb[:, k * C2:(k + 1) * C2], in_=w1bd_d[k])
            w2sb = consts.tile([P, K * P], bf16)
            for k in range(K):
                nc.scalar.dma_start(out=w2sb[:, k * P:(k + 1) * P], in_=w2bd_d[k])
            xlT = consts.tile([S1, R], bf16)
            nc.scalar.dma_start(out=xlT[:], in_=xlocT_d[:])
            b1r = consts.tile([P, C2], f32)
            nc.scalar.dma_start(out=b1r[:], in_=b1r_d[:])
            b2r = consts.tile([P, C2], f32)
            nc.scalar.dma_start(out=b2r[:], in_=b2r_d[:])
            fb1r = consts.tile([B, M1], f32)
            nc.scalar.dma_start(out=fb1r[:], in_=fb1r_d[:])
            fb2r = consts.tile([B, M2], f32)
            nc.scalar.dma_start(out=fb2r[:], in_=fb2r_d[:])
            fb3r = consts.tile([B, M3], f32)
            nc.scalar.dma_start(out=fb3r[:], in_=fb3r_d[:])
            fw3sb = consts.tile([M2, M3], f32r)
            nc.scalar.dma_start(out=fw3sb[:], in_=fw3_d[:])

            def at_sl(mc, j):
                return atsb[:, mc * R + j * P:mc * R + (j + 1) * P]

            def emit_body(rep):
                # =============================================================
                # conv1 — node-major states [P, S1], at stationary
                # =============================================================
                out1 = []
                for j in range(NJ):
                    t = accp.tile([P, C2], f32, tag="acc", name=f"out1_{rep}_{j}")
                    nc.vector.memset(t[:], 0.0)
                    out1.append(t)

                def conv1_feature(k, get_lhsT):
                    """out1[j] += lhsT_j.T @ w1bd[k]; lhsT_j: [S1, P] bf16."""
                    for j in range(NJ):
                        f_ps = psp.tile(
                            [P, C2], f32, tag="psff", bufs=1, name=f"c1f_{rep}_{k}_{j}"
                        )
                        nc.tensor.matmul(
                            f_ps[:], get_lhsT(j), w1sb[:, k * C2:(k + 1) * C2],
                            start=True, stop=True,
                        )
                        nc.vector.tensor_add(out1[j][:], out1[j][:], f_ps[:])

                def conv1_tr(dst, k):
                    """Build transposed locals [S1, P] for the feature matmul."""
                    tts = []
                    for j in range(NJ):
                        psT = psp.tile(
                            [S1, P], bf16, tag="psf", bufs=2, name=f"c1T_{rep}_{k}_{j}"
                        )
                        nc.tensor.transpose(psT[:], dst[j][:], identb[:])
                        tt = ttp.tile([S1, P], bf16, tag="tt1", name=f"c1tt_{rep}_{k}_{j}")
                        nc.vector.tensor_copy(tt[:], psT[:])
                        tts.append(tt)
                    return tts

                # T0 = x: full node-major copy + local tiles
                tf1 = tf1p.tile([P, MC * S1], bf16, tag="tf1", name=f"tf1_{rep}_t0")
                nc.scalar.dma_start(
                    out=tf1.rearrange("p (mc s) -> p mc s", s=S1),
                    in_=x2dp_d.rearrange("(mc p) s -> p mc s", p=P),
                )
                tlA = []
                for j in range(NJ):
                    t = tl1p.tile([P, S1], bf16, tag="tl1", name=f"tlA_{rep}_{j}")
                    nc.scalar.dma_start(out=t[:], in_=xloc_d[j * P:(j + 1) * P, :])
                    tlA.append(t)
                tlB = [
                    tl1p.tile([P, S1], bf16, tag="tl1", name=f"tlB_{rep}_{j}")
                    for j in range(NJ)
                ]
                conv1_feature(0, lambda j: xlT[:, j * P:(j + 1) * P])

                src1 = x2dp_d
                for k in range(1, K):
                    # apply: T*_local = a @ T_{k-1}, two j-halves of 4 psum tiles
                    if k > 1:
                        tf1 = tf1p.tile(
                            [P, MC * S1], bf16, tag="tf1", name=f"tf1_{rep}_{k}"
                        )
                        nc.scalar.dma_start(
                            out=tf1.rearrange("p (mc s) -> p mc s", s=S1),
                            in_=src1.rearrange("(mc p) s -> p mc s", p=P),
                        )
                    dst = tlB if k % 2 == 1 else tlA
                    for h in range(2):
                        ps1 = [
                            psp.tile(
                                [P, S1], f32, tag="psa", bufs=4,
                                name=f"c1ps_{rep}_{k}_{h}_{j}",
                            )
                            for j in range(HJ)
                        ]
                        for mc in range(MC):
                            tfc = tf1[:, mc * S1:(mc + 1) * S1]
                            for jj in range(HJ):
                                nc.tensor.matmul(
                                    ps1[jj][:],
                                    at_sl(mc, h * HJ + jj),
                                    tfc,
                                    start=(mc == 0),
                                    stop=(mc == MC - 1),
                                )
                        # combine: T_k = 2 a T_{k-1} - T_{k-2} (bf16 out)
                        for jj in range(HJ):
                            j = h * HJ + jj
                            if k == 1:
                                nc.vector.tensor_copy(dst[j][:], ps1[jj][:])
                            else:
                                nc.vector.scalar_tensor_tensor(
                                    dst[j][:], ps1[jj][:], 2.0, dst[j][:],
                                    Alu.mult, Alu.subtract,
                                )
                    # gather T_k for the next application
                    if k < K - 1:
                        cc_in = drp.tile([R, S1], bf16, tag="cc1i", name=f"cc1i_{rep}_{k}")
                        for j in range(NJ):
                            nc.sync.dma_start(
                                out=cc_in[j * P:(j + 1) * P, :], in_=dst[j][:]
                            )
                        cc_out = drp.tile(
                            [N, S1], bf16, tag="cc1o", addr_space="Shared",
                            name=f"cc1o_{rep}_{k}",
                        )
                        nc.gpsimd.collective_compute(
                            "AllGather", Alu.bypass, replica_groups=RG,
                            ins=[cc_in.opt()], outs=[cc_out.opt()],
                        )
                        src1 = cc_out
                    conv1_feature(k, (lambda ts: (lambda j: ts[j][:]))(conv1_tr(dst, k)))

                # conv1 epilogue: h = relu(out1 + b1) -> bf16; gather h
                hb = []
                for j in range(NJ):
                    nc.vector.tensor_add(out1[j][:], out1[j][:], b1r[:])
                    nc.vector.tensor_relu(out1[j][:], out1[j][:])
                    t = tl2p.tile([P, C2], bf16, tag="tl2", name=f"hb_{rep}_{j}")
                    nc.vector.tensor_copy(t[:], out1[j][:])
                    hb.append(t)
                cc_hi = drp.tile([R, C2], bf16, tag="cc2i", name=f"cc_hi_{rep}")
                for j in range(NJ):
                    nc.sync.dma_start(out=cc_hi[j * P:(j + 1) * P, :], in_=hb[j][:])
                cc_h = drp.tile(
                    [N, C2], bf16, tag="cc2o", addr_space="Shared", name=f"cc_h_{rep}"
                )
                nc.gpsimd.collective_compute(
                    "AllGather", Alu.bypass, replica_groups=RG,
                    ins=[cc_hi.opt()], outs=[cc_h.opt()],
                )

                # =============================================================
                # conv2 — states [P, C2] bf16, pipelined j-halves
                # =============================================================
                out2 = []
                for j in range(NJ):
                    t = accp.tile([P, C2], f32, tag="acc", name=f"out2_{rep}_{j}")
                    nc.vector.memset(t[:], 0.0)
                    out2.append(t)

                def conv2_feature(tk_set, k, js):
                    """out2[j] += sum_c T_k[j][:, c].T @ w2bd[k] (block-diag)."""
                    for j in js:
                        f_ps = psp.tile(
                            [P, C2], f32, tag="psff", bufs=1,
                            name=f"c2f_{rep}_{k}_{j}",
                        )
                        for c in range(4):
                            tt_ps = psp.tile(
                                [P, P], bf16, tag="psf", bufs=2,
                                name=f"c2T_{rep}_{k}_{j}_{c}",
                            )
                            nc.tensor.transpose(
                                tt_ps[:], tk_set[j][:, c * P:(c + 1) * P], identb[:]
                            )
                            tt = ttp.tile(
                                [P, P], bf16, tag="tt2", name=f"c2tt_{rep}_{k}_{j}_{c}"
                            )
                            nc.vector.tensor_copy(tt[:], tt_ps[:])
                            nc.tensor.matmul(
                                f_ps[:, c * P:(c + 1) * P],
                                tt[:],
                                w2sb[:, k * P:(k + 1) * P],
                                start=True,
                                stop=True,
                            )
                        nc.vector.tensor_add(out2[j][:], out2[j][:], f_ps[:])

                tlB2 = [
                    tl2p.tile([P, C2], bf16, tag="tl2", name=f"tlB2_{rep}_{j}")
                    for j in range(NJ)
                ]
                conv2_feature(hb, 0, range(NJ))  # T0 = h (local rows in hb)

                # gathered T chunks: chunk mc lives in the half-gather of
                # core mc//8, half (mc%8)//4, at rows (mc//8)*512+(mc%4)*128
                srcs = [cc_h, cc_h]

                def tft_src(k, mc):
                    if k == 1:
                        return cc_h[mc * P:(mc + 1) * P, :]
                    c, rem = divmod(mc, NJ)
                    hh, loc = divmod(rem, HJ)
                    off = c * (R // 2) + loc * P
                    return srcs[hh][off:off + P, :]

                for k in range(1, K):
                    dst = tlB2 if k % 2 == 1 else hb
                    new_srcs = [None, None]
                    for h in range(2):
                        ps_g = [
                            psp.tile(
                                [P, C2], f32, tag="psa", bufs=4,
                                name=f"c2g_{rep}_{k}_{h}_{j}",
                            )
                            for j in range(HJ)
                        ]
                        for mc in range(MC):
                            tft = tf2p.tile(
                                [P, C2], bf16, tag="tf2", name=f"tf2_{rep}_{k}_{h}_{mc}"
                            )
                            nc.scalar.dma_start(out=tft[:], in_=tft_src(k, mc))
                            for jj in range(HJ):
                                nc.tensor.matmul(
                                    ps_g[jj][:],
                                    at_sl(mc, h * HJ + jj),
                                    tft[:],
                                    start=(mc == 0),
                                    stop=(mc == MC - 1),
                                )
                        for jj in range(HJ):
                            j = h * HJ + jj
                            if k == 1:
                                nc.vector.tensor_copy(dst[j][:], ps_g[jj][:])
                            else:
                                nc.vector.scalar_tensor_tensor(
                                    dst[j][:], ps_g[jj][:], 2.0, dst[j][:],
                                    Alu.mult, Alu.subtract,
                                )
                        if k < K - 1:
                            cc_in = drp.tile(
                                [R // 2, C2], bf16, tag="cc2i",
                                name=f"cc2i_{rep}_{k}_{h}",
                            )
                            for jj in range(HJ):
                                nc.sync.dma_start(
                                    out=cc_in[jj * P:(jj + 1) * P, :],
                                    in_=dst[h * HJ + jj][:],
                                )
                            cc_out = drp.tile(
                                [N // 2, C2], bf16, tag="cc2o", addr_space="Shared",
                                name=f"cc2o_{rep}_{k}_{h}",
                            )
                            nc.gpsimd.collective_compute(
                                "AllGather", Alu.bypass, replica_groups=RG,
                                ins=[cc_in.opt()], outs=[cc_out.opt()],
                            )
                            new_srcs[h] = cc_out
                        conv2_feature(dst, k, range(h * HJ, (h + 1) * HJ))
                    srcs = new_srcs

                # conv2 epilogue: h2 = relu(out2 + b2) -> bf16 fc1 lhs
                fc_lhs = []
                for j in range(NJ):
                    nc.vector.tensor_add(out2[j][:], out2[j][:], b2r[:])
                    nc.vector.tensor_relu(out2[j][:], out2[j][:])
                    t = tl2p.tile([P, C2], bf16, tag="tl2", name=f"h2b_{rep}_{j}")
                    nc.vector.tensor_copy(t[:], out2[j][:])
                    fc_lhs.append(t)

                # =============================================================
                # fc1 (node-sharded contraction) + AllReduce
                # =============================================================
                fw1v = fw1s_d.rearrange("(j p f) m -> j f p m", p=P, f=F2)
                fc_ps = psp.tile([B, M1], f32, tag="psfc", bufs=1, name=f"fc1_{rep}")
                n_mm = NJ * F2
                i_mm = 0
                for j in range(NJ):
                    lhs_j = fc_lhs[j].rearrange("p (b f) -> p f b", f=F2)
                    for f in range(F2):
                        fwt = fwp.tile(
                            [P, M1], bf16, tag="fw", name=f"fw1_{rep}_{j}_{f}"
                        )
                        eng = nc.sync if f % 2 == 0 else nc.scalar
                        eng.dma_start(out=fwt[:], in_=fw1v[j, f])
                        nc.tensor.matmul(
                            fc_ps[:],
                            lhs_j[:, f, :],
                            fwt[:],
                            start=(i_mm == 0),
                            stop=(i_mm == n_mm - 1),
                        )
                        i_mm += 1

                z = fcp.tile([B, M1], f32, tag="z", name=f"z_{rep}")
                nc.vector.tensor_copy(z[:], fc_ps[:])
                cc_fi = drp.tile([B, M1], f32, tag="ccfi", name=f"ccfi_{rep}")
                nc.sync.dma_start(out=cc_fi[:], in_=z[:])
                cc_fo = drp.tile(
                    [B, M1], f32, tag="ccfo", addr_space="Shared", name=f"ccfo_{rep}"
                )
                nc.gpsimd.collective_compute(
                    "AllReduce", Alu.add, replica_groups=RG,
                    ins=[cc_fi.opt()], outs=[cc_fo.opt()],
                )

                # z1 = relu(fc1 + fb1), padded to 32 partitions for PE transpose
                z1p = fcp.tile([32, M1], f32, tag="z1p", name=f"z1p_{rep}")
                nc.vector.memset(z1p[:], 0.0)
                nc.sync.dma_start(out=z1p[0:B, :], in_=cc_fo[:])
                nc.vector.tensor_add(z1p[0:B, :], z1p[0:B, :], fb1r[:])
                nc.vector.tensor_relu(z1p[0:B, :], z1p[0:B, :])

                # fc2
                fc2_ps = psp.tile([B, M2], f32, tag="psfc", bufs=1, name=f"fc2_{rep}")
                for c in range(4):
                    zt_ps = psp.tile(
                        [P, 32], f32, tag="psf", bufs=2, name=f"ztps_{rep}_{c}"
                    )
                    nc.tensor.transpose(
                        zt_ps[:], z1p[:, c * P:(c + 1) * P], identf[0:32, 0:32]
                    )
                    zt = fcp.tile([P, 32], f32r, tag="zt", name=f"zt_{rep}_{c}")
                    nc.vector.tensor_copy(zt[:], zt_ps[:])
                    fwt2 = fcp.tile([P, M2], f32r, tag="fw2t", name=f"fw2t_{rep}_{c}")
                    nc.sync.dma_start(out=fwt2[:], in_=fw2_d[c * P:(c + 1) * P, :])
                    nc.tensor.matmul(
                        fc2_ps[:],
                        zt[:, 0:B],
                        fwt2[:],
                        start=(c == 0),
                        stop=(c == 3),
                    )
                z2p = fcp.tile([32, M2], f32, tag="z2p", name=f"z2p_{rep}")
                nc.vector.memset(z2p[:], 0.0)
                nc.vector.tensor_copy(z2p[0:B, :], fc2_ps[:])
                nc.vector.tensor_add(z2p[0:B, :], z2p[0:B, :], fb2r[:])
                nc.vector.tensor_relu(z2p[0:B, :], z2p[0:B, :])

                # fc3
                z3t_ps = psp.tile([P, 32], f32, tag="psf", bufs=2, name=f"z3tps_{rep}")
                nc.tensor.transpose(z3t_ps[:], z2p[:], identf[0:32, 0:32])
                z3t = fcp.tile([P, 32], f32r, tag="z3t", name=f"z3t_{rep}")
                nc.vector.tensor_copy(z3t[:], z3t_ps[:])
                fc3_ps = psp.tile([B, M3], f32, tag="psf", bufs=2, name=f"fc3_{rep}")
                nc.tensor.matmul(
                    fc3_ps[:], z3t[:, 0:B], fw3sb[:], start=True, stop=True
                )
                s = fcp.tile([B, M3], f32, tag="s", name=f"s_{rep}")
                nc.vector.tensor_copy(s[:], fc3_ps[:])
                nc.vector.tensor_add(s[:], s[:], fb3r[:])

                # softmax over the last dim (M3 = 2)
                mx = fcp.tile([B, 1], f32, tag="mx", name=f"mx_{rep}")
                nc.vector.reduce_max(mx[:], s[:], axis=mybir.AxisListType.X)
                nc.vector.tensor_scalar_mul(mx[:], mx[:], -1.0)
                nc.scalar.activation(s[:], s[:], Act.Exp, bias=mx[:, 0:1])
                sm = fcp.tile([B, 1], f32, tag="sm", name=f"sm_{rep}")
                nc.vector.reduce_sum(sm[:], s[:], axis=mybir.AxisListType.X)
                nc.vector.reciprocal(sm[:], sm[:])
                nc.vector.tensor_scalar_mul(s[:], s[:], sm[:, 0:1])
                nc.sync.dma_start(out=out_d[:], in_=s[:])

            for _rep in range(REPEAT):
                emit_body(_rep)

    nc.compile()
    return nc


def prepare_inputs(x, a, w1, b1, w2, b2, fw1, fb1, fw2, fb2, fw3, fb3):
    """Shard + re-layout the full model inputs into 8 per-core input maps."""
    import ml_dtypes

    bf = ml_dtypes.bfloat16

    x = np.asarray(x, np.float32)
    a = np.asarray(a, np.float32)
    w1 = np.asarray(w1, np.float32)
    w2 = np.asarray(w2, np.float32)
    fw1 = np.asarray(fw1, np.float32)

    # node-major [N, B*F_IN]
    x2d = x.transpose(1, 0, 2).reshape(N, S1)
    x2d_c = x2d.astype(bf)

    w1bd = np.zeros((K, S1, C2), np.float32)
    for b in range(B):
        w1bd[:, b * F_IN:(b + 1) * F_IN, b * F1:(b + 1) * F1] = w1
    w1bd_c = w1bd.astype(bf)
    w2bd = np.zeros((K, P, P), np.float32)
    for q in range(4):
        w2bd[:, q * F1:(q + 1) * F1, q * F2:(q + 1) * F2] = w2
    w2bd_c = w2bd.astype(bf)

    b1r = np.broadcast_to(np.tile(np.asarray(b1, np.float32), B), (P, C2)).copy()
    b2r = np.broadcast_to(np.tile(np.asarray(b2, np.float32), B), (P, C2)).copy()
    fb1r = np.broadcast_to(np.asarray(fb1, np.float32), (B, M1)).copy()
    fb2r = np.broadcast_to(np.asarray(fb2, np.float32), (B, M2)).copy()
    fb3r = np.broadcast_to(np.asarray(fb3, np.float32), (B, M3)).copy()
    fw2_c = np.asarray(fw2, np.float32)
    fw3_c = np.asarray(fw3, np.float32)

    fw1_3 = fw1.reshape(N, F2, M1)

    in_maps = []
    for i in range(NCORES):
        r0 = i * R
        at_i = np.ascontiguousarray(a[r0:r0 + R, :].T).astype(bf)
        xloc = np.ascontiguousarray(x2d[r0:r0 + R]).astype(bf)
        xlocT = np.ascontiguousarray(x2d[r0:r0 + R].T).astype(bf)
        fw1s = np.ascontiguousarray(fw1_3[r0:r0 + R].reshape(R * F2, M1)).astype(bf)
        in_maps.append(
            {
                "at": at_i,
                "x2dp": x2d_c,
                "xloc": xloc,
                "xlocT": xlocT,
                "w1bd": w1bd_c,
                "w2bd": w2bd_c,
                "fw1s": fw1s,
                "fw2": fw2_c,
                "fw3": fw3_c,
                "b1r": b1r,
                "b2r": b2r,
                "fb1r": fb1r,
                "fb2r": fb2r,
                "fb3r": fb3r,
            }
        )
    return in_maps


def kernel(**inputs) -> np.ndarray:
    from concourse.bass_utils import run_bass_kernel_spmd

    key = "v2"
    if key not in _CACHE:
        _CACHE[key] = build_kernel()
    nc = _CACHE[key]

    in_maps = prepare_inputs(**inputs)
    res = run_bass_kernel_spmd(nc, in_maps, core_ids=list(range(NCORES)))
    return np.asarray(res.results[0]["out"], np.float32)


if __name__ == "__main__":
    import importlib.util

    spec = importlib.util.spec_from_file_location(
        "reference", os.path.join(os.path.dirname(__file__), "reference.py")
    )
    ref = importlib.util.module_from_spec(spec)
    spec.loader.exec_module(ref)
    inputs = {k: np.asarray(v) for k, v in ref.setup_inputs().items()}
    out = kernel(**inputs)
    print(out)


# revision 10
# speedup vs baseline: 5.0478x; 1.3176x over previous
"""Trainium2 Bass kernel: 2-layer Chebyshev graph conv (K=5) + 3-layer MLP head.

Distribution over 8 NeuronCores (v2 — SBUF-resident adjacency):
  - The adjacency a [8192, 8192] is row-sharded; core i owns graph rows
    [1024*i, 1024*(i+1)), passed host-transposed AND host-cast to bf16
    (at: [8192, 1024]) so the contraction dim m is the partition dim of the
    stationary matmul operand.  The whole 16 MiB shard is loaded into SBUF
    ONCE and reused by all 8 Chebyshev applications (4 in conv1, 4 in
    conv2) — the baseline re-streamed 32 MiB fp32 per application.
  - Chebyshev states are node-major bf16.  Per application, each core
    computes its local 1024 rows (psum accumulation over 64 m-chunks with
    the at-slices stationary), then AllGathers them for the next
    application.  conv2 runs in two j-halves (4 PSUM banks each) so the
    combine/spill/gather of one half pipelines under the other's matmuls.
  - conv outputs accumulate in f32 SBUF via per-j PE transposes +
    block-diagonal weight matmuls (weights bf16).
  - fc1 is contraction(row)-sharded over nodes, weights streamed as bf16;
    partial [16, 512] results are AllReduced, then fc2/fc3/softmax run
    redundantly on every core.
"""

import os
import sys

import numpy as np

for _p in ("/opt/trn_rl_repo", "/root/.axon_site/_ro/trn_rl_repo"):
    if os.path.isdir(_p) and _p not in sys.path:
        sys.path.insert(0, _p)

P = 128          # SBUF partitions
N = 8192         # nodes
B = 16           # batch
F_IN = 2
F1 = 32
F2 = 32
K = 5            # Chebyshev order
NCORES = 8
R = N // NCORES  # nodes per core (1024)
MC = N // P      # m-chunks (64)
NJ = R // P      # local n-chunks (8)
S1 = B * F_IN    # conv1 state width (32)
C2 = B * F1      # 512
M1, M2, M3 = 512, 128, 2
HJ = NJ // 2     # j-chunks per half (4)

REPEAT = int(os.environ.get("KER_REPEAT", "1"))    # emit the body N times (timing A/B)

_CACHE = {}


def build_kernel(dt_graph=None, dt_fc1=None, repeat=None):
    from concourse import bacc, mybir, tile
    from concourse.masks import make_identity

    REPEAT = repeat if repeat is not None else globals()["REPEAT"]

    dt = mybir.dt
    f32 = dt.float32
    f32r = dt.float32r
    bf16 = dt.bfloat16
    Alu = mybir.AluOpType
    Act = mybir.ActivationFunctionType
    RG = [list(range(NCORES))]

    nc = bacc.Bacc(
        "TRN2",
        target_bir_lowering=False,
        debug=False,
        enable_asserts=False,
        num_devices=NCORES,
    )

    # ------------------------- DRAM I/O -------------------------
    at_d = nc.dram_tensor("at", [N, R], bf16, kind="ExternalInput").ap()
    x2dp_d = nc.dram_tensor("x2dp", [N, S1], bf16, kind="ExternalInput").ap()
    xloc_d = nc.dram_tensor("xloc", [R, S1], bf16, kind="ExternalInput").ap()
    xlocT_d = nc.dram_tensor("xlocT", [S1, R], bf16, kind="ExternalInput").ap()
    w1bd_d = nc.dram_tensor("w1bd", [K, S1, C2], bf16, kind="ExternalInput").ap()
    w2bd_d = nc.dram_tensor("w2bd", [K, P, P], bf16, kind="ExternalInput").ap()
    fw1s_d = nc.dram_tensor("fw1s", [R * F2, M1], bf16, kind="ExternalInput").ap()
    fw2_d = nc.dram_tensor("fw2", [M1, M2], f32r, kind="ExternalInput").ap()
    fw3_d = nc.dram_tensor("fw3", [M2, M3], f32r, kind="ExternalInput").ap()
    b1r_d = nc.dram_tensor("b1r", [P, C2], f32, kind="ExternalInput").ap()
    b2r_d = nc.dram_tensor("b2r", [P, C2], f32, kind="ExternalInput").ap()
    fb1r_d = nc.dram_tensor("fb1r", [B, M1], f32, kind="ExternalInput").ap()
    fb2r_d = nc.dram_tensor("fb2r", [B, M2], f32, kind="ExternalInput").ap()
    fb3r_d = nc.dram_tensor("fb3r", [B, M3], f32, kind="ExternalInput").ap()
    out_d = nc.dram_tensor("out", [B, M3], f32, kind="ExternalOutput").ap()

    with tile.TileContext(nc) as tc:
        with (
            tc.tile_pool(name="consts", bufs=1) as consts,
            tc.tile_pool(name="tf1p", bufs=2) as tf1p,
            tc.tile_pool(name="tf2p", bufs=6) as tf2p,
            tc.tile_pool(name="accp", bufs=8) as accp,
            tc.tile_pool(name="tl1p", bufs=16) as tl1p,
            tc.tile_pool(name="tl2p", bufs=16) as tl2p,
            tc.tile_pool(name="ttp", bufs=4) as ttp,
            tc.tile_pool(name="fwp", bufs=6) as fwp,
            tc.tile_pool(name="fcp", bufs=1) as fcp,
            tc.tile_pool(name="psum", bufs=1, space="PSUM") as psp,
            tc.tile_pool(name="dram", bufs=3, space="DRAM") as drp,
        ):
            # ------------------------- constants -------------------------
            identb = consts.tile([P, P], bf16)
            make_identity(nc, identb)
            identf = consts.tile([P, P], f32)
            make_identity(nc, identf)
            # the full a^T shard, SBUF-resident: [m-partition, mc, r]
            atsb = consts.tile([P, MC * R], bf16)
            nc.sync.dma_start(
                out=atsb.rearrange("p (mc r) -> p mc r", r=R),
                in_=at_d.rearrange("(mc p) r -> p mc r", p=P),
            )
            w1sb = consts.tile([S1, K * C2], bf16)
            for k in range(K):
                nc.scalar.dma_start(out=w1sb[:, k * C2:(k + 1) * C2], in_=w1bd_d[k])
            w2sb = consts.tile([P, K * P], bf16)
            for k in range(K):
                nc.scalar.dma_start(out=w2sb[:, k * P:(k + 1) * P], in_=w2bd_d[k])
            xlT = consts.tile([S1, R], bf16)
            nc.scalar.dma_start(out=xlT[:], in_=xlocT_d[:])
            b1r = consts.tile([P, C2], f32)
            nc.scalar.dma_start(out=b1r[:], in_=b1r_d[:])
            b2r = consts.tile([P, C2], f32)
            nc.scalar.dma_start(out=b2r[:], in_=b2r_d[:])
            fb1r = consts.tile([B, M1], f32)
            nc.scalar.dma_start(out=fb1r[:], in_=fb1r_d[:])
            fb2r = consts.tile([B, M2], f32)
            nc.scalar.dma_start(out=fb2r[:], in_=fb2r_d[:])
            fb3r = consts.tile([B, M3], f32)
            nc.scalar.dma_start(out=fb3r[:], in_=fb3r_d[:])
            fw3sb = consts.tile([M2, M3], f32r)
            nc.scalar.dma_start(out=fw3sb[:], in_=fw3_d[:])

            def at_sl(mc, j):
                return atsb[:, mc * R + j * P:mc * R + (j + 1) * P]

            def emit_body(rep):
                # =============================================================
                # conv1 — node-major states [P, S1], at stationary
                # =============================================================
                out1 = []
                for j in range(NJ):
                    t = accp.tile([P, C2], f32, tag="acc", name=f"out1_{rep}_{j}")
                    nc.vector.memset(t[:], 0.0)
                    out1.append(t)

                def conv1_feature(k, get_lhsT):
                    """out1[j] += lhsT_j.T @ w1bd[k]; lhsT_j: [S1, P] bf16."""
                    for j in range(NJ):
                        f_ps = psp.tile(
                            [P, C2], f32, tag="psff", bufs=1, name=f"c1f_{rep}_{k}_{j}"
                        )
                        nc.tensor.matmul(
                            f_ps[:], get_lhsT(j), w1sb[:, k * C2:(k + 1) * C2],
                            start=True, stop=True,
                        )
                        nc.vector.tensor_add(out1[j][:], out1[j][:], f_ps[:])

                def conv1_tr(dst, k):
                    """Build transposed locals [S1, P] for the feature matmul."""
                    tts = []
                    for j in range(NJ):
                        psT = psp.tile(
                            [S1, P], bf16, tag="psf", bufs=2, name=f"c1T_{rep}_{k}_{j}"
                        )
                        nc.tensor.transpose(psT[:], dst[j][:], identb[:])
                        tt = ttp.tile([S1, P], bf16, tag="tt1", name=f"c1tt_{rep}_{k}_{j}")
                        nc.vector.tensor_copy(tt[:], psT[:])
                        tts.append(tt)
                    return tts

                # T0 = x: full node-major copy + local tiles
                tf1 = tf1p.tile([P, MC * S1], bf16, tag="tf1", name=f"tf1_{rep}_t0")
                nc.scalar.dma_start(
                    out=tf1.rearrange("p (mc s) -> p mc s", s=S1),
                    in_=x2dp_d.rearrange("(mc p) s -> p mc s", p=P),
                )
                tlA = []
                for j in range(NJ):
                    t = tl1p.tile([P, S1], bf16, tag="tl1", name=f"tlA_{rep}_{j}")
                    nc.scalar.dma_start(out=t[:], in_=xloc_d[j * P:(j + 1) * P, :])
                    tlA.append(t)
                tlB = [
                    tl1p.tile([P, S1], bf16, tag="tl1", name=f"tlB_{rep}_{j}")
                    for j in range(NJ)
                ]
                conv1_feature(0, lambda j: xlT[:, j * P:(j + 1) * P])

                src1 = x2dp_d
                for k in range(1, K):
                    # apply: T*_local = a @ T_{k-1}, two j-halves of 4 psum tiles
                    if k > 1:
                        tf1 = tf1p.tile(
                            [P, MC * S1], bf16, tag="tf1", name=f"tf1_{rep}_{k}"
                        )
                        nc.scalar.dma_start(
                            out=tf1.rearrange("p (mc s) -> p mc s", s=S1),
                            in_=src1.rearrange("(mc p) s -> p mc s", p=P),
                        )
                    dst = tlB if k % 2 == 1 else tlA
                    for h in range(2):
                        ps1 = [
                            psp.tile(
                                [P, S1], f32, tag="psa", bufs=4,
                                name=f"c1ps_{rep}_{k}_{h}_{j}",
                            )
                            for j in range(HJ)
                        ]
                        for mc in range(MC):
                            tfc = tf1[:, mc * S1:(mc + 1) * S1]
                            for jj in range(HJ):
                                nc.tensor.matmul(
                                    ps1[jj][:],
                                    at_sl(mc, h * HJ + jj),
                                    tfc,
                                    start=(mc == 0),
                                    stop=(mc == MC - 1),
                                )
                        # combine: T_k = 2 a T_{k-1} - T_{k-2} (bf16 out)
                        for jj in range(HJ):
                            j = h * HJ + jj
                            if k == 1:
                                nc.vector.tensor_copy(dst[j][:], ps1[jj][:])
                            else:
                                nc.vector.scalar_tensor_tensor(
                                    dst[j][:], ps1[jj][:], 2.0, dst[j][:],
                                    Alu.mult, Alu.subtract,
                                )
                    # gather T_k for the next application
                    if k < K - 1:
                        cc_in = drp.tile([R, S1], bf16, tag="cc1i", name=f"cc1i_{rep}_{k}")
                        for j in range(NJ):
                            nc.sync.dma_start(
                                out=cc_in[j * P:(j + 1) * P, :], in_=dst[j][:]
                            )
                        cc_out = drp.tile(
                            [N, S1], bf16, tag="cc1o", addr_space="Shared",
                            name=f"cc1o_{rep}_{k}",
                        )
                        nc.gpsimd.collective_compute(
                            "AllGather", Alu.bypass, replica_groups=RG,
                            ins=[cc_in.opt()], outs=[cc_out.opt()],
                        )
                        src1 = cc_out
                    conv1_feature(k, (lambda ts: (lambda j: ts[j][:]))(conv1_tr(dst, k)))

                # conv1 epilogue: h = relu(out1 + b1) -> bf16; gather h in
                # j-halves so the first gather fires while the second half's
                # epilogue still runs
                hb = []
                srcs = [None, None]
                for h in range(2):
                    for jj in range(HJ):
                        j = h * HJ + jj
                        nc.vector.tensor_add(out1[j][:], out1[j][:], b1r[:])
                        nc.vector.tensor_relu(out1[j][:], out1[j][:])
                        t = tl2p.tile([P, C2], bf16, tag="tl2", name=f"hb_{rep}_{j}")
                        nc.vector.tensor_copy(t[:], out1[j][:])
                        hb.append(t)
                    cc_hi = drp.tile(
                        [R // 2, C2], bf16, tag="cc2i", name=f"cc_hi_{rep}_{h}"
                    )
                    for jj in range(HJ):
                        nc.sync.dma_start(
                            out=cc_hi[jj * P:(jj + 1) * P, :], in_=hb[h * HJ + jj][:]
                        )
                    cc_ho = drp.tile(
                        [N // 2, C2], bf16, tag="cc2o", addr_space="Shared",
                        name=f"cc_ho_{rep}_{h}",
                    )
                    nc.gpsimd.collective_compute(
                        "AllGather", Alu.bypass, replica_groups=RG,
                        ins=[cc_hi.opt()], outs=[cc_ho.opt()],
                    )
                    srcs[h] = cc_ho

                # =============================================================
                # conv2 — states [P, C2] bf16, pipelined j-halves
                # =============================================================
                out2 = []
                for j in range(NJ):
                    t = accp.tile([P, C2], f32, tag="acc", name=f"out2_{rep}_{j}")
                    nc.vector.memset(t[:], 0.0)
                    out2.append(t)

                def conv2_feature(tk_set, k, js):
                    """out2[j] += sum_c T_k[j][:, c].T @ w2bd[k] (block-diag)."""
                    for j in js:
                        f_ps = psp.tile(
                            [P, C2], f32, tag="psff", bufs=1,
                            name=f"c2f_{rep}_{k}_{j}",
                        )
                        for c in range(4):
                            tt_ps = psp.tile(
                                [P, P], bf16, tag="psf", bufs=2,
                                name=f"c2T_{rep}_{k}_{j}_{c}",
                            )
                            nc.tensor.transpose(
                                tt_ps[:], tk_set[j][:, c * P:(c + 1) * P], identb[:]
                            )
                            tt = ttp.tile(
                                [P, P], bf16, tag="tt2", name=f"c2tt_{rep}_{k}_{j}_{c}"
                            )
                            nc.vector.tensor_copy(tt[:], tt_ps[:])
                            nc.tensor.matmul(
                                f_ps[:, c * P:(c + 1) * P],
                                tt[:],
                                w2sb[:, k * P:(k + 1) * P],
                                start=True,
                                stop=True,
                            )
                        nc.vector.tensor_add(out2[j][:], out2[j][:], f_ps[:])

                tlB2 = [
                    tl2p.tile([P, C2], bf16, tag="tl2", name=f"tlB2_{rep}_{j}")
                    for j in range(NJ)
                ]
                conv2_feature(hb, 0, range(NJ))  # T0 = h (local rows in hb)

                # gathered T chunks: chunk mc lives in the half-gather of
                # core mc//8, half (mc%8)//4, at rows (mc//8)*512+(mc%4)*128
                def tft_src(mc):
                    c, rem = divmod(mc, NJ)
                    hh, loc = divmod(rem, HJ)
                    off = c * (R // 2) + loc * P
                    return srcs[hh][off:off + P, :]

                def apply_half(k, h, dst, extra_pe=None):
                    ps_g = [
                        psp.tile(
                            [P, C2], f32, tag="psa", bufs=4,
                            name=f"c2g_{rep}_{k}_{h}_{j}",
                        )
                        for j in range(HJ)
                    ]
                    for mc in range(MC):
                        tft = tf2p.tile(
                            [P, C2], bf16, tag="tf2", name=f"tf2_{rep}_{k}_{h}_{mc}"
                        )
                        nc.scalar.dma_start(out=tft[:], in_=tft_src(mc))
                        for jj in range(HJ):
                            nc.tensor.matmul(
                                ps_g[jj][:],
                                at_sl(mc, h * HJ + jj),
                                tft[:],
                                start=(mc == 0),
                                stop=(mc == MC - 1),
                            )
                        if extra_pe is not None:
                            extra_pe(mc)
                    for jj in range(HJ):
                        j = h * HJ + jj
                        if k == 1:
                            nc.vector.tensor_copy(dst[j][:], ps_g[jj][:])
                        else:
                            nc.vector.scalar_tensor_tensor(
                                dst[j][:], ps_g[jj][:], 2.0, dst[j][:],
                                Alu.mult, Alu.subtract,
                            )

                for k in range(1, K - 1):
                    dst = tlB2 if k % 2 == 1 else hb
                    new_srcs = [None, None]
                    for h in range(2):
                        apply_half(k, h, dst)
                        cc_in = drp.tile(
                            [R // 2, C2], bf16, tag="cc2i",
                            name=f"cc2i_{rep}_{k}_{h}",
                        )
                        for jj in range(HJ):
                            nc.sync.dma_start(
                                out=cc_in[jj * P:(jj + 1) * P, :],
                                in_=dst[h * HJ + jj][:],
                            )
                        cc_out = drp.tile(
                            [N // 2, C2], bf16, tag="cc2o", addr_space="Shared",
                            name=f"cc2o_{rep}_{k}_{h}",
                        )
                        nc.gpsimd.collective_compute(
                            "AllGather", Alu.bypass, replica_groups=RG,
                            ins=[cc_in.opt()], outs=[cc_out.opt()],
                        )
                        new_srcs[h] = cc_out
                        conv2_feature(dst, k, range(h * HJ, (h + 1) * HJ))
                    srcs = new_srcs

                # ---- last k-step (k = K-1, no gather).  The first half's
                # feature/epilogue/fc1 work is interleaved INTO the second
                # half's apply on PE, so the fw1 stream (32 MiB) flows at PE
                # consumption pace under the h1 matmuls ----
                k = K - 1
                dst = hb  # K-1 = 4 is even
                apply_half(k, 0, dst)
                conv2_feature(dst, k, range(0, HJ))

                fw1v = fw1s_d.rearrange("(j p f) m -> j f p m", p=P, f=F2)
                fc_ps = psp.tile([B, M1], f32, tag="psfc", bufs=1, name=f"fc1_{rep}")
                n_mm = NJ * F2
                fc_lhs = []

                def epi_j(j):
                    nc.vector.tensor_add(out2[j][:], out2[j][:], b2r[:])
                    nc.vector.tensor_relu(out2[j][:], out2[j][:])
                    t = tl2p.tile([P, C2], bf16, tag="tl2", name=f"h2b_{rep}_{j}")
                    nc.vector.tensor_copy(t[:], out2[j][:])
                    fc_lhs.append(t)

                for jj in range(HJ):
                    epi_j(jj)

                state = {"i": 0}

                def emit_fc1(j, f):
                    i = state["i"]
                    lhs_j = fc_lhs[j].rearrange("p (b f) -> p f b", f=F2)
                    fwt = fwp.tile([P, M1], bf16, tag="fw", name=f"fw1_{rep}_{j}_{f}")
                    eng = nc.sync if f % 2 == 0 else nc.scalar
                    eng.dma_start(out=fwt[:], in_=fw1v[j, f])
                    nc.tensor.matmul(
                        fc_ps[:],
                        lhs_j[:, f, :],
                        fwt[:],
                        start=(i == 0),
                        stop=(i == n_mm - 1),
                    )
                    state["i"] = i + 1

                jobs = [(j, f) for j in range(HJ) for f in range(F2)]
                START_MC = 8
                SPAN = MC - START_MC
                done = {"n": 0}

                def extra_pe(mc):
                    if mc < START_MC:
                        return
                    want = (mc - START_MC + 1) * len(jobs) // SPAN
                    while done["n"] < want:
                        j, f = jobs[done["n"]]
                        emit_fc1(j, f)
                        done["n"] += 1

                apply_half(k, 1, dst, extra_pe)
                conv2_feature(dst, k, range(HJ, NJ))
                for jj in range(HJ, NJ):
                    epi_j(jj)
                for j in range(HJ, NJ):
                    for f in range(F2):
                        emit_fc1(j, f)

                z = fcp.tile([B, M1], f32, tag="z", name=f"z_{rep}")
                nc.vector.tensor_copy(z[:], fc_ps[:])
                cc_fi = drp.tile([B, M1], f32, tag="ccfi", name=f"ccfi_{rep}")
                nc.sync.dma_start(out=cc_fi[:], in_=z[:])
                cc_fo = drp.tile(
                    [B, M1], f32, tag="ccfo", addr_space="Shared", name=f"ccfo_{rep}"
                )
                nc.gpsimd.collective_compute(
                    "AllReduce", Alu.add, replica_groups=RG,
                    ins=[cc_fi.opt()], outs=[cc_fo.opt()],
                )

                # z1 = relu(fc1 + fb1), padded to 32 partitions for PE transpose
                z1p = fcp.tile([32, M1], f32, tag="z1p", name=f"z1p_{rep}")
                nc.vector.memset(z1p[:], 0.0)
                nc.sync.dma_start(out=z1p[0:B, :], in_=cc_fo[:])
                nc.vector.tensor_add(z1p[0:B, :], z1p[0:B, :], fb1r[:])
                nc.vector.tensor_relu(z1p[0:B, :], z1p[0:B, :])

                # fc2
                fc2_ps = psp.tile([B, M2], f32, tag="psfc", bufs=1, name=f"fc2_{rep}")
                for c in range(4):
                    zt_ps = psp.tile(
                        [P, 32], f32, tag="psf", bufs=2, name=f"ztps_{rep}_{c}"
                    )
                    nc.tensor.transpose(
                        zt_ps[:], z1p[:, c * P:(c + 1) * P], identf[0:32, 0:32]
                    )
                    zt = fcp.tile([P, 32], f32r, tag="zt", name=f"zt_{rep}_{c}")
                    nc.vector.tensor_copy(zt[:], zt_ps[:])
                    fwt2 = fcp.tile([P, M2], f32r, tag="fw2t", name=f"fw2t_{rep}_{c}")
                    nc.sync.dma_start(out=fwt2[:], in_=fw2_d[c * P:(c + 1) * P, :])
                    nc.tensor.matmul(
                        fc2_ps[:],
                        zt[:, 0:B],
                        fwt2[:],
                        start=(c == 0),
                        stop=(c == 3),
                    )
                z2p = fcp.tile([32, M2], f32, tag="z2p", name=f"z2p_{rep}")
                nc.vector.memset(z2p[:], 0.0)
                nc.vector.tensor_copy(z2p[0:B, :], fc2_ps[:])
                nc.vector.tensor_add(z2p[0:B, :], z2p[0:B, :], fb2r[:])
                nc.vector.tensor_relu(z2p[0:B, :], z2p[0:B, :])

                # fc3
                z3t_ps = psp.tile([P, 32], f32, tag="psf", bufs=2, name=f"z3tps_{rep}")
                nc.tensor.transpose(z3t_ps[:], z2p[:], identf[0:32, 0:32])
                z3t = fcp.tile([P, 32], f32r, tag="z3t", name=f"z3t_{rep}")
                nc.vector.tensor_copy(z3t[:], z3t_ps[:])
                fc3_ps = psp.tile([B, M3], f32, tag="psf", bufs=2, name=f"fc3_{rep}")
                nc.tensor.matmul(
                    fc3_ps[:], z3t[:, 0:B], fw3sb[:], start=True, stop=True
                )
                s = fcp.tile([B, M3], f32, tag="s", name=f"s_{rep}")
                nc.vector.tensor_copy(s[:], fc3_ps[:])
                nc.vector.tensor_add(s[:], s[:], fb3r[:])

                # softmax over the last dim (M3 = 2)
                mx = fcp.tile([B, 1], f32, tag="mx", name=f"mx_{rep}")
                nc.vector.reduce_max(mx[:], s[:], axis=mybir.AxisListType.X)
                nc.vector.tensor_scalar_mul(mx[:], mx[:], -1.0)
                nc.scalar.activation(s[:], s[:], Act.Exp, bias=mx[:, 0:1])
                sm = fcp.tile([B, 1], f32, tag="sm", name=f"sm_{rep}")
                nc.vector.reduce_sum(sm[:], s[:], axis=mybir.AxisListType.X)
                nc.vector.reciprocal(sm[:], sm[:])
                nc.vector.tensor_scalar_mul(s[:], s[:], sm[:, 0:1])
                nc.sync.dma_start(out=out_d[:], in_=s[:])

            for _rep in range(REPEAT):
                emit_body(_rep)

    nc.compile()
    return nc


def prepare_inputs(x, a, w1, b1, w2, b2, fw1, fb1, fw2, fb2, fw3, fb3):
    """Shard + re-layout the full model inputs into 8 per-core input maps."""
    import ml_dtypes

    bf = ml_dtypes.bfloat16

    x = np.asarray(x, np.float32)
    a = np.asarray(a, np.float32)
    w1 = np.asarray(w1, np.float32)
    w2 = np.asarray(w2, np.float32)
    fw1 = np.asarray(fw1, np.float32)

    # node-major [N, B*F_IN]
    x2d = x.transpose(1, 0, 2).reshape(N, S1)
    x2d_c = x2d.astype(bf)

    w1bd = np.zeros((K, S1, C2), np.float32)
    for b in range(B):
        w1bd[:, b * F_IN:(b + 1) * F_IN, b * F1:(b + 1) * F1] = w1
    w1bd_c = w1bd.astype(bf)
    w2bd = np.zeros((K, P, P), np.float32)
    for q in range(4):
        w2bd[:, q * F1:(q + 1) * F1, q * F2:(q + 1) * F2] = w2
    w2bd_c = w2bd.astype(bf)

    b1r = np.broadcast_to(np.tile(np.asarray(b1, np.float32), B), (P, C2)).copy()
    b2r = np.broadcast_to(np.tile(np.asarray(b2, np.float32), B), (P, C2)).copy()
    fb1r = np.broadcast_to(np.asarray(fb1, np.float32), (B, M1)).copy()
    fb2r = np.broadcast_to(np.asarray(fb2, np.float32), (B, M2)).copy()
    fb3r = np.broadcast_to(np.asarray(fb3, np.float32), (B, M3)).copy()
    fw2_c = np.asarray(fw2, np.float32)
    fw3_c = np.asarray(fw3, np.float32)

    fw1_3 = fw1.reshape(N, F2, M1)

    in_maps = []
    for i in range(NCORES):
        r0 = i * R
        at_i = np.ascontiguousarray(a[r0:r0 + R, :].T).astype(bf)
        xloc = np.ascontiguousarray(x2d[r0:r0 + R]).astype(bf)
        xlocT = np.ascontiguousarray(x2d[r0:r0 + R].T).astype(bf)
        fw1s = np.ascontiguousarray(fw1_3[r0:r0 + R].reshape(R * F2, M1)).astype(bf)
        in_maps.append(
            {
                "at": at_i,
                "x2dp": x2d_c,
                "xloc": xloc,
                "xlocT": xlocT,
                "w1bd": w1bd_c,
                "w2bd": w2bd_c,
                "fw1s": fw1s,
                "fw2": fw2_c,
                "fw3": fw3_c,
                "b1r": b1r,
                "b2r": b2r,
                "fb1r": fb1r,
                "fb2r": fb2r,
                "fb3r": fb3r,
            }
        )
    return in_maps


def kernel(**inputs) -> np.ndarray:
    from concourse.bass_utils import run_bass_kernel_spmd

    key = "v2"
    if key not in _CACHE:
        _CACHE[key] = build_kernel()
    nc = _CACHE[key]

    in_maps = prepare_inputs(**inputs)
    res = run_bass_kernel_spmd(nc, in_maps, core_ids=list(range(NCORES)))
    return np.asarray(res.results[0]["out"], np.float32)


if __name__ == "__main__":
    import importlib.util

    spec = importlib.util.spec_from_file_location(
        "reference", os.path.join(os.path.dirname(__file__), "reference.py")
    )
    ref = importlib.util.module_from_spec(spec)
    spec.loader.exec_module(ref)
    inputs = {k: np.asarray(v) for k, v in ref.setup_inputs().items()}
    out = kernel(**inputs)
    print(out)
